# revision 43
# baseline (speedup 1.0000x reference)
"""Fused Linear + GroupNorm + Hardtanh kernel for Trainium2 (8 NeuronCores).

Problem: out = clip(groupnorm(x @ W.T + b, 32 groups), -2, 2), with
x [65536, 512] fp32, W [1024, 512] fp32, gamma=1/beta=0.

Strategy (data-parallel over the 8 cores, 8192 rows each):
 - Host removes the per-group mean from the weights (W' = W - mean_g W)
   and bias (c = b - mean_g b), so PSUM holds exactly y - mean(y) - c
   with zero on-device mean statistics.
 - The constant c row is added while converting PSUM to fp16 ("t"),
   either on the Pool engine (one tensor_tensor Add) or via K=1 PE
   matmuls seeding PSUM (INJECT_MODE).
 - Variance: ScalarE squares t, VectorE does the segmented 32-wide sum
   (the one op stuck at DVE 1x), ScalarE sqrt(Q/32+eps), VectorE fast
   reciprocal; ScalarE octet-duplicates rstd to fp16 so the apply
   multiply runs in a DVE fast mode.
 - Apply: VectorE multiply t by the octet-broadcast rstd, in-place fp16
   clip, fp16 store (host upcasts).
"""
import sys

sys.path.insert(0, "/opt/trn_rl_repo")

import numpy as np

M_FULL, K, N = 65536, 512, 1024
NG, GS = 32, 32
EPS = 1e-5
HT = 2.0
N_CORES = 8
KT = K // 128  # 4 k-tiles
CHUNK = 1024  # x.T columns loaded per DMA chunk (8 m-tiles)
INJECT_MODE = "pe"  # "pe": K=1 fp16 matmuls seed PSUM with c
                    # "pe8": K=1 fp8 DoubleRow matmuls
GFOLD = True  # Pool engine pre-folds the 32-wide group sum to 16-wide
RSQRT = False  # single ACT Rsqrt instead of sqrt + DVE reciprocal

_custom_ops = {}


def _register_custom_ops():
    """Fused scale+clip DVE op (idempotent registration)."""
    if _custom_ops:
        return _custom_ops
    import concourse.dve_ops as dve_ops
    from concourse.dve_spec import Spec, Src0, Src1, C0, C1, minn, maxx, \
        lower, _has_src1
    from concourse.dve_uop import DveOpSpec

    def register(name, spec):
        if name in dve_ops._SUB_OPCODE_FOR_NAME:
            return next(o for o in dve_ops.OPS if o.name == name)
        row = max(dve_ops._SUB_OPCODE_FOR_NAME.values()) + 1
        assert row < 0x20
        op = dve_ops.DveOp(name, spec, subdim=False, uops_sha={})
        dve_ops.OPS.append(op)
        dve_ops._SUB_OPCODE_FOR_NAME[name] = row
        dve_ops.CUSTOM_DVE_SPECS[name] = spec
        for ver in ("v3", "v4"):
            uops = lower(spec, ver=ver)
            op.uops_sha[ver] = DveOpSpec(
                name=name, opcode=row, uops=uops,
                rd1_en=_has_src1(spec)).sha(ver)
        return op

    _custom_ops["apply"] = register("APPLY_SCALE_CLIP_ANT", Spec(
        body=minn(maxx(Src0 * Src1, C0), C1),
        reference=lambda in0, in1, s0, s1, imm2: np.minimum(
            np.maximum(in0.astype(np.float32) * in1, s0), s1)))
    return _custom_ops



def build(m_loc: int, apply_affine: bool):
    import concourse.bass as bass
    import concourse.mybir as mybir
    import concourse.tile as tile
    from concourse import bacc
    from contextlib import ExitStack

    ops = _register_custom_ops()
    f32 = mybir.dt.float32
    f16 = mybir.dt.float16
    Alu = mybir.AluOpType
    n_tiles = m_loc // 128
    chunk = min(CHUNK, m_loc)
    tpc = chunk // 128  # m-tiles per x.T chunk

    f8 = mybir.dt.float8e4
    nc = bacc.Bacc()
    xt_d = nc.dram_tensor("xt", [K, m_loc], f16, kind="ExternalInput")
    wt_d = nc.dram_tensor("wt", [K, N], f16, kind="ExternalInput")
    if INJECT_MODE == "pe8":
        crow_d = nc.dram_tensor("crow8", [1, 2 * N], f8,
                                kind="ExternalInput")
        ones_d = nc.dram_tensor("ones8", [1, 256], f8, kind="ExternalInput")
    else:
        crow_d = nc.dram_tensor("crow", [1, N], f16, kind="ExternalInput")
    if apply_affine:
        gam_d = nc.dram_tensor("gam", [128, N], f32, kind="ExternalInput")
        bet_d = nc.dram_tensor("bet", [128, N], f32, kind="ExternalInput")
    out_d = nc.dram_tensor("out", [m_loc, N], f16, kind="ExternalOutput")

    with tile.TileContext(nc) as tc, ExitStack() as ctx:
        const = ctx.enter_context(tc.tile_pool(name="const", bufs=1))
        xpool = ctx.enter_context(tc.tile_pool(name="xts", bufs=2 * KT))
        pp = ctx.enter_context(tc.tile_pool(name="pp", bufs=4, space="PSUM"))
        mid = ctx.enter_context(tc.tile_pool(name="mid", bufs=12))
        small = ctx.enter_context(tc.tile_pool(name="small", bufs=20))
        outp = ctx.enter_context(tc.tile_pool(name="outp", bufs=10))

        # --- resident constants (first k-tile of W first, then the first
        # x chunk, so the PE can start after two DMAs land) ---
        wt_sb = []
        for kt in range(KT):
            w = const.tile([128, N], f16, tag=f"wt{kt}")
            wt_sb.append(w)
        nc.sync.dma_start(out=wt_sb[0][:], in_=wt_d[0:128, :])
        xts0 = []
        chunk0 = min(CHUNK, m_loc)
        for kt in range(KT):
            t = xpool.tile([128, chunk0], f16, tag="xts")
            nc.sync.dma_start(out=t[:], in_=xt_d[kt * 128:(kt + 1) * 128,
                                                 0:chunk0])
            xts0.append(t)
        for kt in range(1, KT):
            nc.sync.dma_start(out=wt_sb[kt][:],
                              in_=wt_d[kt * 128:(kt + 1) * 128, :])
        eps_sb = const.tile([128, 1], f32, tag="eps")
        nc.vector.memset(eps_sb[:], EPS)
        if INJECT_MODE == "pe8":
            crow_sb = const.tile([1, 2 * N], f8, tag="crow8")
            nc.sync.dma_start(out=crow_sb[:], in_=crow_d[:])
            ones_sb = const.tile([1, 256], f8, tag="ones8")
            nc.sync.dma_start(out=ones_sb[:], in_=ones_d[:])
        else:
            crow_sb = const.tile([1, N], f16, tag="crow")
            nc.sync.dma_start(out=crow_sb[:], in_=crow_d[:])
            ones_sb = const.tile([1, 128], f16, tag="ones")
            nc.vector.memset(ones_sb[:], 1.0)
        if apply_affine:
            gam_sb = const.tile([128, N], f32, tag="gam")
            nc.sync.dma_start(out=gam_sb[:], in_=gam_d[:])
            bet_sb = const.tile([128, N], f32, tag="bet")
            nc.sync.dma_start(out=bet_sb[:], in_=bet_d[:])

        state = {}
        xts_cur = [None]

        def emit_main(mt):
            sc, loc = divmod(mt, tpc)
            if loc == 0:
                if mt == 0:
                    xts_cur[0] = xts0
                else:
                    xts = []
                    for kt in range(KT):
                        t = xpool.tile([128, chunk], f16, tag="xts")
                        nc.sync.dma_start(
                            out=t[:],
                            in_=xt_d[kt * 128:(kt + 1) * 128,
                                     sc * chunk:(sc + 1) * chunk])
                        xts.append(t)
                    xts_cur[0] = xts
            xts = xts_cur[0]
            ph = pp.tile([128, N], f32, tag="ph")
            for kt in range(KT):
                lhsT = xts[kt][:, loc * 128:(loc + 1) * 128]
                for h in range(2):
                    nc.tensor.matmul(
                        ph[:, 512 * h:512 * (h + 1)], lhsT,
                        wt_sb[kt][:, 512 * h:512 * (h + 1)],
                        start=(kt == 0), stop=False)
            # bias-minus-groupmean row injected last (K=1 rank-1 update)
            if INJECT_MODE == "pe8":
                for h in range(2):
                    nc.tensor.matmul(
                        ph[:, 512 * h:512 * (h + 1)],
                        ones_sb[:].rearrange("p (two f) -> p two f", two=2),
                        crow_sb[:, 1024 * h:1024 * (h + 1)].rearrange(
                            "p (two f) -> p two f", two=2),
                        start=False, stop=True,
                        perf_mode=mybir.MatmulPerfMode.DoubleRow)
            else:
                for h in range(2):
                    nc.tensor.matmul(ph[:, 512 * h:512 * (h + 1)],
                                     ones_sb[:],
                                     crow_sb[:, 512 * h:512 * (h + 1)],
                                     start=False, stop=True)
            state[mt] = ph

        def emit_epi(mt):
            ph = state.pop(mt)
            ysq = mid.tile([128, N], f16, tag="ysq")
            nc.scalar.square(ysq[:], ph[:])
            Q = small.tile([128, NG], f16, tag="Q")
            with nc.allow_low_precision("groupsum of 32 fp16 squares"):
                if GFOLD:
                    # Pool engine folds 32-wide groups to 8-wide first
                    y2 = mid.tile([128, N // 2], f16, tag="y2")
                    lo = ysq[:, 0:N]
                    hi = ysq[:, GS // 2:N]
                    in0 = bass.AP(tensor=lo.tensor, offset=lo.offset,
                                  ap=[lo.ap[0], [GS, NG], [1, GS // 2]])
                    in1 = bass.AP(tensor=hi.tensor, offset=hi.offset,
                                  ap=[hi.ap[0], [GS, NG], [1, GS // 2]])
                    nc.gpsimd.tensor_tensor(out=y2[:], in0=in0, in1=in1,
                                            op=Alu.add)
                    nc.vector.tensor_reduce(
                        out=Q[:],
                        in_=y2[:].rearrange("p (g e) -> p g e", e=GS // 2),
                        axis=mybir.AxisListType.X, op=Alu.add)
                else:
                    nc.vector.tensor_reduce(
                        out=Q[:],
                        in_=ysq[:].rearrange("p (g e) -> p g e", e=GS),
                        axis=mybir.AxisListType.X, op=Alu.add)
            # rstd = rsqrt(Q/32 + eps) in one ACT op. The wrapper refuses
            # Rsqrt (low-precision LUT) but tolerance here is 2e-2, so emit
            # the instruction directly; scale+bias fold in.
            r = small.tile([128, NG], f32, tag="r")
            if RSQRT:
                ins_ = [nc.scalar.lower_ap(Q[:]),
                        nc.scalar.lower_ap(eps_sb[:]),
                        mybir.ImmediateValue(dtype=f32, value=1.0 / GS),
                        mybir.ImmediateValue(dtype=f32, value=0.0)]
                nc.scalar.add_instruction(mybir.InstActivation(
                    name=nc.get_next_instruction_name(),
                    func=mybir.ActivationFunctionType.Rsqrt,
                    ins=ins_, outs=[nc.scalar.lower_ap(r[:])]))
            else:
                s = small.tile([128, NG], f32, tag="s")
                nc.scalar.activation(
                    out=s[:], in_=Q[:],
                    func=mybir.ActivationFunctionType.Sqrt,
                    bias=eps_sb[:], scale=1.0 / GS)
                nc.vector.reciprocal_approx_fast(r[:], s[:])
            o = outp.tile([128, N], f16, tag="o")
            lo, hi = (-1e30, 1e30) if apply_affine else (-HT, HT)
            for h in range(2):
                rh = r[:, 16 * h:16 * (h + 1)]
                rb = bass.AP(tensor=rh.tensor, offset=rh.offset,
                             ap=[rh.ap[0], rh.ap[1], [0, GS]])
                nc.vector._custom_dve(
                    ops["apply"],
                    out=o[:, 512 * h:512 * (h + 1)].rearrange(
                        "p (g e) -> p g e", e=GS),
                    in0=ph[:, 512 * h:512 * (h + 1)].rearrange(
                        "p (g e) -> p g e", e=GS),
                    in1=rb, s0=lo, s1=hi)
            if apply_affine:
                nc.vector.tensor_mul(o[:], o[:], gam_sb[:])
                nc.vector.tensor_add(o[:], o[:], bet_sb[:])
                nc.vector.tensor_scalar(
                    out=o[:], in0=o[:], scalar1=-HT, scalar2=HT,
                    op0=Alu.max, op1=Alu.min)
            nc.sync.dma_start(out=out_d[mt * 128:(mt + 1) * 128, :], in_=o[:])

        for mt in range(n_tiles):
            emit_main(mt)
            if mt >= 1:
                emit_epi(mt - 1)
        emit_epi(n_tiles - 1)

    nc.finalize()
    return nc


def _prep_host(weight, bias):
    f16 = np.float16
    w = weight.astype(np.float64).reshape(NG, GS, K)
    wp = (w - w.mean(axis=1, keepdims=True)).reshape(N, K)
    wt_h = np.ascontiguousarray(wp.T.astype(f16))  # [K, N]
    b = bias.astype(np.float64).reshape(NG, GS)
    c = (b - b.mean(axis=1, keepdims=True)).reshape(N)
    if INJECT_MODE == "pe8":
        import ml_dtypes
        f8 = ml_dtypes.float8_e4m3fn
        # DoubleRow layout [K=1, two, F]: sub-row 0 carries c, sub-row 1 is 0
        crow_h = np.zeros((1, 2 * N), dtype=f8)
        crow_h[0, 0:512] = c[0:512].astype(f8)
        crow_h[0, 1024:1536] = c[512:1024].astype(f8)
        ones_h = np.zeros((1, 256), dtype=f8)
        ones_h[0, 0:128] = f8(1.0)
        return wt_h, crow_h, ones_h
    crow_h = np.ascontiguousarray(c[None, :].astype(f16))
    return wt_h, crow_h, None


def run(x, weight, bias, gamma, beta, m_loc=None, trace=False):
    from concourse.bass_utils import run_bass_kernel_spmd

    f16 = np.float16
    x = np.asarray(x, dtype=np.float32)
    weight = np.asarray(weight, dtype=np.float32)
    bias = np.asarray(bias, dtype=np.float32)
    gamma = np.asarray(gamma, dtype=np.float32)
    beta = np.asarray(beta, dtype=np.float32)

    m_total = x.shape[0]
    if m_loc is None:
        m_loc = m_total // N_CORES
    assert m_total == m_loc * N_CORES

    apply_affine = not (np.all(gamma == 1.0) and np.all(beta == 0.0))
    nc = build(m_loc, apply_affine)
    wt_h, crow_h, ones_h = _prep_host(weight, bias)

    in_maps = []
    for c in range(N_CORES):
        m = {
            "xt": np.ascontiguousarray(
                x[c * m_loc:(c + 1) * m_loc, :].T.astype(f16)),
            "wt": wt_h,
        }
        if INJECT_MODE == "pe8":
            m["crow8"] = crow_h
            m["ones8"] = ones_h
        else:
            m["crow"] = crow_h
        if apply_affine:
            m["gam"] = np.ascontiguousarray(np.broadcast_to(gamma, (128, N)))
            m["bet"] = np.ascontiguousarray(np.broadcast_to(beta, (128, N)))
        in_maps.append(m)

    res = run_bass_kernel_spmd(nc, in_maps, list(range(N_CORES)), trace=trace)
    out = np.concatenate([res.results[c]["out"] for c in range(N_CORES)],
                         axis=0).astype(np.float32)
    return out, res


def kernel(x, weight, bias, gamma, beta):
    out, _ = run(x, weight, bias, gamma, beta)
    return out


# revision 44
# speedup vs baseline: 1.0113x; 1.0113x over previous
"""Fused Linear + GroupNorm + Hardtanh kernel for Trainium2 (8 NeuronCores).

Problem: out = clip(groupnorm(x @ W.T + b, 32 groups), -2, 2), with
x [65536, 512] fp32, W [1024, 512] fp32, gamma=1/beta=0.

Strategy (data-parallel over the 8 cores, 8192 rows each):
 - Host removes the per-group mean from the weights (W' = W - mean_g W)
   and bias (c = b - mean_g b), so PSUM holds exactly y - mean(y) - c
   with zero on-device mean statistics.
 - The constant c row is added while converting PSUM to fp16 ("t"),
   either on the Pool engine (one tensor_tensor Add) or via K=1 PE
   matmuls seeding PSUM (INJECT_MODE).
 - Variance: ScalarE squares t, VectorE does the segmented 32-wide sum
   (the one op stuck at DVE 1x), ScalarE sqrt(Q/32+eps), VectorE fast
   reciprocal; ScalarE octet-duplicates rstd to fp16 so the apply
   multiply runs in a DVE fast mode.
 - Apply: VectorE multiply t by the octet-broadcast rstd, in-place fp16
   clip, fp16 store (host upcasts).
"""
import sys

sys.path.insert(0, "/opt/trn_rl_repo")

import numpy as np

M_FULL, K, N = 65536, 512, 1024
NG, GS = 32, 32
EPS = 1e-5
HT = 2.0
N_CORES = 8
KT = K // 128  # 4 k-tiles
CHUNK = 1024  # x.T columns loaded per DMA chunk (8 m-tiles)
INJECT_MODE = "pe"  # "pe": K=1 fp16 matmuls seed PSUM with c
                    # "pe8": K=1 fp8 DoubleRow matmuls
GFOLD = True  # Pool engine pre-folds the 32-wide group sum to 16-wide
RSQRT = False  # single ACT Rsqrt instead of sqrt + DVE reciprocal

_custom_ops = {}


def _register_custom_ops():
    """Fused scale+clip DVE op (idempotent registration)."""
    if _custom_ops:
        return _custom_ops
    import concourse.dve_ops as dve_ops
    from concourse.dve_spec import Spec, Src0, Src1, C0, C1, minn, maxx, \
        lower, _has_src1
    from concourse.dve_uop import DveOpSpec

    def register(name, spec):
        if name in dve_ops._SUB_OPCODE_FOR_NAME:
            return next(o for o in dve_ops.OPS if o.name == name)
        row = max(dve_ops._SUB_OPCODE_FOR_NAME.values()) + 1
        assert row < 0x20
        op = dve_ops.DveOp(name, spec, subdim=False, uops_sha={})
        dve_ops.OPS.append(op)
        dve_ops._SUB_OPCODE_FOR_NAME[name] = row
        dve_ops.CUSTOM_DVE_SPECS[name] = spec
        for ver in ("v3", "v4"):
            uops = lower(spec, ver=ver)
            op.uops_sha[ver] = DveOpSpec(
                name=name, opcode=row, uops=uops,
                rd1_en=_has_src1(spec)).sha(ver)
        return op

    _custom_ops["apply"] = register("APPLY_SCALE_CLIP_ANT", Spec(
        body=minn(maxx(Src0 * Src1, C0), C1),
        reference=lambda in0, in1, s0, s1, imm2: np.minimum(
            np.maximum(in0.astype(np.float32) * in1, s0), s1)))
    return _custom_ops



def build(m_loc: int, apply_affine: bool):
    import concourse.bass as bass
    import concourse.mybir as mybir
    import concourse.tile as tile
    from concourse import bacc
    from contextlib import ExitStack

    ops = _register_custom_ops()
    f32 = mybir.dt.float32
    f16 = mybir.dt.float16
    Alu = mybir.AluOpType
    n_tiles = m_loc // 128
    chunk = min(CHUNK, m_loc)
    tpc = chunk // 128  # m-tiles per x.T chunk

    f8 = mybir.dt.float8e4
    nc = bacc.Bacc()
    xt_d = nc.dram_tensor("xt", [K, m_loc], f16, kind="ExternalInput")
    wt_d = nc.dram_tensor("wt", [K, N], f16, kind="ExternalInput")
    if INJECT_MODE == "pe8":
        crow_d = nc.dram_tensor("crow8", [1, 2 * N], f8,
                                kind="ExternalInput")
        ones_d = nc.dram_tensor("ones8", [1, 256], f8, kind="ExternalInput")
    else:
        crow_d = nc.dram_tensor("crow", [1, N], f16, kind="ExternalInput")
    if apply_affine:
        gam_d = nc.dram_tensor("gam", [128, N], f32, kind="ExternalInput")
        bet_d = nc.dram_tensor("bet", [128, N], f32, kind="ExternalInput")
    out_d = nc.dram_tensor("out", [m_loc, N], f16, kind="ExternalOutput")

    with tile.TileContext(nc) as tc, ExitStack() as ctx:
        const = ctx.enter_context(tc.tile_pool(name="const", bufs=1))
        xpool = ctx.enter_context(tc.tile_pool(name="xts", bufs=2 * KT))
        pp = ctx.enter_context(tc.tile_pool(name="pp", bufs=4, space="PSUM"))
        mid = ctx.enter_context(tc.tile_pool(name="mid", bufs=8))
        small = ctx.enter_context(tc.tile_pool(name="small", bufs=12))
        outp = ctx.enter_context(tc.tile_pool(name="outp", bufs=6))

        # --- resident constants (first k-tile of W first, then the first
        # x chunk, so the PE can start after two DMAs land) ---
        wt_sb = []
        for kt in range(KT):
            w = const.tile([128, N], f16, tag=f"wt{kt}")
            wt_sb.append(w)
        nc.sync.dma_start(out=wt_sb[0][:], in_=wt_d[0:128, :])
        xts0 = []
        chunk0 = min(CHUNK, m_loc)
        for kt in range(KT):
            t = xpool.tile([128, chunk0], f16, tag="xts")
            nc.sync.dma_start(out=t[:], in_=xt_d[kt * 128:(kt + 1) * 128,
                                                 0:chunk0])
            xts0.append(t)
        for kt in range(1, KT):
            nc.sync.dma_start(out=wt_sb[kt][:],
                              in_=wt_d[kt * 128:(kt + 1) * 128, :])
        eps_sb = const.tile([128, 1], f32, tag="eps")
        nc.vector.memset(eps_sb[:], EPS)
        if INJECT_MODE == "pe8":
            crow_sb = const.tile([1, 2 * N], f8, tag="crow8")
            nc.sync.dma_start(out=crow_sb[:], in_=crow_d[:])
            ones_sb = const.tile([1, 256], f8, tag="ones8")
            nc.sync.dma_start(out=ones_sb[:], in_=ones_d[:])
        else:
            crow_sb = const.tile([1, N], f16, tag="crow")
            nc.sync.dma_start(out=crow_sb[:], in_=crow_d[:])
            ones_sb = const.tile([1, 128], f16, tag="ones")
            nc.vector.memset(ones_sb[:], 1.0)
        if apply_affine:
            gam_sb = const.tile([128, N], f32, tag="gam")
            nc.sync.dma_start(out=gam_sb[:], in_=gam_d[:])
            bet_sb = const.tile([128, N], f32, tag="bet")
            nc.sync.dma_start(out=bet_sb[:], in_=bet_d[:])

        state = {}
        xts_cur = [None]

        def emit_main(mt):
            sc, loc = divmod(mt, tpc)
            if loc == 0:
                if mt == 0:
                    xts_cur[0] = xts0
                else:
                    xts = []
                    for kt in range(KT):
                        t = xpool.tile([128, chunk], f16, tag="xts")
                        nc.sync.dma_start(
                            out=t[:],
                            in_=xt_d[kt * 128:(kt + 1) * 128,
                                     sc * chunk:(sc + 1) * chunk])
                        xts.append(t)
                    xts_cur[0] = xts
            xts = xts_cur[0]
            ph = pp.tile([128, N], f32, tag="ph")
            for kt in range(KT):
                lhsT = xts[kt][:, loc * 128:(loc + 1) * 128]
                for h in range(2):
                    nc.tensor.matmul(
                        ph[:, 512 * h:512 * (h + 1)], lhsT,
                        wt_sb[kt][:, 512 * h:512 * (h + 1)],
                        start=(kt == 0), stop=False)
            # bias-minus-groupmean row injected last (K=1 rank-1 update)
            if INJECT_MODE == "pe8":
                for h in range(2):
                    nc.tensor.matmul(
                        ph[:, 512 * h:512 * (h + 1)],
                        ones_sb[:].rearrange("p (two f) -> p two f", two=2),
                        crow_sb[:, 1024 * h:1024 * (h + 1)].rearrange(
                            "p (two f) -> p two f", two=2),
                        start=False, stop=True,
                        perf_mode=mybir.MatmulPerfMode.DoubleRow)
            else:
                for h in range(2):
                    nc.tensor.matmul(ph[:, 512 * h:512 * (h + 1)],
                                     ones_sb[:],
                                     crow_sb[:, 512 * h:512 * (h + 1)],
                                     start=False, stop=True)
            state[mt] = ph

        def emit_epi(mt):
            ph = state.pop(mt)
            ysq = mid.tile([128, N], f16, tag="ysq")
            nc.scalar.square(ysq[:], ph[:])
            Q = small.tile([128, NG], f16, tag="Q")
            with nc.allow_low_precision("groupsum of 32 fp16 squares"):
                if GFOLD:
                    # Pool engine folds 32-wide groups to 8-wide first
                    y2 = mid.tile([128, N // 2], f16, tag="y2")
                    lo = ysq[:, 0:N]
                    hi = ysq[:, GS // 2:N]
                    in0 = bass.AP(tensor=lo.tensor, offset=lo.offset,
                                  ap=[lo.ap[0], [GS, NG], [1, GS // 2]])
                    in1 = bass.AP(tensor=hi.tensor, offset=hi.offset,
                                  ap=[hi.ap[0], [GS, NG], [1, GS // 2]])
                    nc.gpsimd.tensor_tensor(out=y2[:], in0=in0, in1=in1,
                                            op=Alu.add)
                    nc.vector.tensor_reduce(
                        out=Q[:],
                        in_=y2[:].rearrange("p (g e) -> p g e", e=GS // 2),
                        axis=mybir.AxisListType.X, op=Alu.add)
                else:
                    nc.vector.tensor_reduce(
                        out=Q[:],
                        in_=ysq[:].rearrange("p (g e) -> p g e", e=GS),
                        axis=mybir.AxisListType.X, op=Alu.add)
            # rstd = rsqrt(Q/32 + eps) in one ACT op. The wrapper refuses
            # Rsqrt (low-precision LUT) but tolerance here is 2e-2, so emit
            # the instruction directly; scale+bias fold in.
            r = small.tile([128, NG], f32, tag="r")
            if RSQRT:
                ins_ = [nc.scalar.lower_ap(Q[:]),
                        nc.scalar.lower_ap(eps_sb[:]),
                        mybir.ImmediateValue(dtype=f32, value=1.0 / GS),
                        mybir.ImmediateValue(dtype=f32, value=0.0)]
                nc.scalar.add_instruction(mybir.InstActivation(
                    name=nc.get_next_instruction_name(),
                    func=mybir.ActivationFunctionType.Rsqrt,
                    ins=ins_, outs=[nc.scalar.lower_ap(r[:])]))
            else:
                s = small.tile([128, NG], f32, tag="s")
                nc.scalar.activation(
                    out=s[:], in_=Q[:],
                    func=mybir.ActivationFunctionType.Sqrt,
                    bias=eps_sb[:], scale=1.0 / GS)
                nc.vector.reciprocal_approx_fast(r[:], s[:])
            o = outp.tile([128, N], f16, tag="o")
            lo, hi = (-1e30, 1e30) if apply_affine else (-HT, HT)
            for h in range(2):
                rh = r[:, 16 * h:16 * (h + 1)]
                rb = bass.AP(tensor=rh.tensor, offset=rh.offset,
                             ap=[rh.ap[0], rh.ap[1], [0, GS]])
                nc.vector._custom_dve(
                    ops["apply"],
                    out=o[:, 512 * h:512 * (h + 1)].rearrange(
                        "p (g e) -> p g e", e=GS),
                    in0=ph[:, 512 * h:512 * (h + 1)].rearrange(
                        "p (g e) -> p g e", e=GS),
                    in1=rb, s0=lo, s1=hi)
            if apply_affine:
                nc.vector.tensor_mul(o[:], o[:], gam_sb[:])
                nc.vector.tensor_add(o[:], o[:], bet_sb[:])
                nc.vector.tensor_scalar(
                    out=o[:], in0=o[:], scalar1=-HT, scalar2=HT,
                    op0=Alu.max, op1=Alu.min)
            nc.sync.dma_start(out=out_d[mt * 128:(mt + 1) * 128, :], in_=o[:])

        for mt in range(n_tiles):
            emit_main(mt)
            if mt >= 1:
                emit_epi(mt - 1)
        emit_epi(n_tiles - 1)

    nc.finalize()
    return nc


def _prep_host(weight, bias):
    f16 = np.float16
    w = weight.astype(np.float64).reshape(NG, GS, K)
    wp = (w - w.mean(axis=1, keepdims=True)).reshape(N, K)
    wt_h = np.ascontiguousarray(wp.T.astype(f16))  # [K, N]
    b = bias.astype(np.float64).reshape(NG, GS)
    c = (b - b.mean(axis=1, keepdims=True)).reshape(N)
    if INJECT_MODE == "pe8":
        import ml_dtypes
        f8 = ml_dtypes.float8_e4m3fn
        # DoubleRow layout [K=1, two, F]: sub-row 0 carries c, sub-row 1 is 0
        crow_h = np.zeros((1, 2 * N), dtype=f8)
        crow_h[0, 0:512] = c[0:512].astype(f8)
        crow_h[0, 1024:1536] = c[512:1024].astype(f8)
        ones_h = np.zeros((1, 256), dtype=f8)
        ones_h[0, 0:128] = f8(1.0)
        return wt_h, crow_h, ones_h
    crow_h = np.ascontiguousarray(c[None, :].astype(f16))
    return wt_h, crow_h, None


def run(x, weight, bias, gamma, beta, m_loc=None, trace=False):
    from concourse.bass_utils import run_bass_kernel_spmd

    f16 = np.float16
    x = np.asarray(x, dtype=np.float32)
    weight = np.asarray(weight, dtype=np.float32)
    bias = np.asarray(bias, dtype=np.float32)
    gamma = np.asarray(gamma, dtype=np.float32)
    beta = np.asarray(beta, dtype=np.float32)

    m_total = x.shape[0]
    if m_loc is None:
        m_loc = m_total // N_CORES
    assert m_total == m_loc * N_CORES

    apply_affine = not (np.all(gamma == 1.0) and np.all(beta == 0.0))
    nc = build(m_loc, apply_affine)
    wt_h, crow_h, ones_h = _prep_host(weight, bias)

    in_maps = []
    for c in range(N_CORES):
        m = {
            "xt": np.ascontiguousarray(
                x[c * m_loc:(c + 1) * m_loc, :].T.astype(f16)),
            "wt": wt_h,
        }
        if INJECT_MODE == "pe8":
            m["crow8"] = crow_h
            m["ones8"] = ones_h
        else:
            m["crow"] = crow_h
        if apply_affine:
            m["gam"] = np.ascontiguousarray(np.broadcast_to(gamma, (128, N)))
            m["bet"] = np.ascontiguousarray(np.broadcast_to(beta, (128, N)))
        in_maps.append(m)

    res = run_bass_kernel_spmd(nc, in_maps, list(range(N_CORES)), trace=trace)
    out = np.concatenate([res.results[c]["out"] for c in range(N_CORES)],
                         axis=0).astype(np.float32)
    return out, res


def kernel(x, weight, bias, gamma, beta):
    out, _ = run(x, weight, bias, gamma, beta)
    return out


# revision 45
# speedup vs baseline: 1.0186x; 1.0072x over previous
"""Fused Linear + GroupNorm + Hardtanh kernel for Trainium2 (8 NeuronCores).

Problem: out = clip(groupnorm(x @ W.T + b, 32 groups), -2, 2), with
x [65536, 512] fp32, W [1024, 512] fp32, gamma=1/beta=0.

Strategy (data-parallel over the 8 cores, 8192 rows each):
 - Host removes the per-group mean from the weights (W' = W - mean_g W)
   and bias (c = b - mean_g b), so PSUM holds exactly y - mean(y) - c
   with zero on-device mean statistics.
 - The constant c row is added while converting PSUM to fp16 ("t"),
   either on the Pool engine (one tensor_tensor Add) or via K=1 PE
   matmuls seeding PSUM (INJECT_MODE).
 - Variance: ScalarE squares t, VectorE does the segmented 32-wide sum
   (the one op stuck at DVE 1x), ScalarE sqrt(Q/32+eps), VectorE fast
   reciprocal; ScalarE octet-duplicates rstd to fp16 so the apply
   multiply runs in a DVE fast mode.
 - Apply: VectorE multiply t by the octet-broadcast rstd, in-place fp16
   clip, fp16 store (host upcasts).
"""
import sys

sys.path.insert(0, "/opt/trn_rl_repo")

import numpy as np

M_FULL, K, N = 65536, 512, 1024
NG, GS = 32, 32
EPS = 1e-5
HT = 2.0
N_CORES = 8
KT = K // 128  # 4 k-tiles
CHUNK = 512  # x.T columns loaded per DMA chunk (4 m-tiles)
INJECT_MODE = "pe"  # "pe": K=1 fp16 matmuls seed PSUM with c
                    # "pe8": K=1 fp8 DoubleRow matmuls
GFOLD = True  # Pool engine pre-folds the 32-wide group sum to 16-wide
RSQRT = False  # single ACT Rsqrt instead of sqrt + DVE reciprocal

_custom_ops = {}


def _register_custom_ops():
    """Fused scale+clip DVE op (idempotent registration)."""
    if _custom_ops:
        return _custom_ops
    import concourse.dve_ops as dve_ops
    from concourse.dve_spec import Spec, Src0, Src1, C0, C1, minn, maxx, \
        lower, _has_src1
    from concourse.dve_uop import DveOpSpec

    def register(name, spec):
        if name in dve_ops._SUB_OPCODE_FOR_NAME:
            return next(o for o in dve_ops.OPS if o.name == name)
        row = max(dve_ops._SUB_OPCODE_FOR_NAME.values()) + 1
        assert row < 0x20
        op = dve_ops.DveOp(name, spec, subdim=False, uops_sha={})
        dve_ops.OPS.append(op)
        dve_ops._SUB_OPCODE_FOR_NAME[name] = row
        dve_ops.CUSTOM_DVE_SPECS[name] = spec
        for ver in ("v3", "v4"):
            uops = lower(spec, ver=ver)
            op.uops_sha[ver] = DveOpSpec(
                name=name, opcode=row, uops=uops,
                rd1_en=_has_src1(spec)).sha(ver)
        return op

    _custom_ops["apply"] = register("APPLY_SCALE_CLIP_ANT", Spec(
        body=minn(maxx(Src0 * Src1, C0), C1),
        reference=lambda in0, in1, s0, s1, imm2: np.minimum(
            np.maximum(in0.astype(np.float32) * in1, s0), s1)))
    return _custom_ops



def build(m_loc: int, apply_affine: bool):
    import concourse.bass as bass
    import concourse.mybir as mybir
    import concourse.tile as tile
    from concourse import bacc
    from contextlib import ExitStack

    ops = _register_custom_ops()
    f32 = mybir.dt.float32
    f16 = mybir.dt.float16
    Alu = mybir.AluOpType
    n_tiles = m_loc // 128
    chunk = min(CHUNK, m_loc)
    tpc = chunk // 128  # m-tiles per x.T chunk

    f8 = mybir.dt.float8e4
    nc = bacc.Bacc()
    xt_d = nc.dram_tensor("xt", [K, m_loc], f16, kind="ExternalInput")
    wt_d = nc.dram_tensor("wt", [K, N], f16, kind="ExternalInput")
    if INJECT_MODE == "pe8":
        crow_d = nc.dram_tensor("crow8", [1, 2 * N], f8,
                                kind="ExternalInput")
        ones_d = nc.dram_tensor("ones8", [1, 256], f8, kind="ExternalInput")
    else:
        crow_d = nc.dram_tensor("crow", [1, N], f16, kind="ExternalInput")
    if apply_affine:
        gam_d = nc.dram_tensor("gam", [128, N], f32, kind="ExternalInput")
        bet_d = nc.dram_tensor("bet", [128, N], f32, kind="ExternalInput")
    out_d = nc.dram_tensor("out", [m_loc, N], f16, kind="ExternalOutput")

    with tile.TileContext(nc) as tc, ExitStack() as ctx:
        const = ctx.enter_context(tc.tile_pool(name="const", bufs=1))
        xpool = ctx.enter_context(tc.tile_pool(name="xts", bufs=2 * KT))
        pp = ctx.enter_context(tc.tile_pool(name="pp", bufs=4, space="PSUM"))
        mid = ctx.enter_context(tc.tile_pool(name="mid", bufs=8))
        small = ctx.enter_context(tc.tile_pool(name="small", bufs=12))
        outp = ctx.enter_context(tc.tile_pool(name="outp", bufs=6))

        # --- resident constants (first k-tile of W first, then the first
        # x chunk, so the PE can start after two DMAs land) ---
        wt_sb = []
        for kt in range(KT):
            w = const.tile([128, N], f16, tag=f"wt{kt}")
            wt_sb.append(w)
        nc.sync.dma_start(out=wt_sb[0][:], in_=wt_d[0:128, :])
        xts0 = []
        chunk0 = min(CHUNK, m_loc)
        for kt in range(KT):
            t = xpool.tile([128, chunk0], f16, tag="xts")
            nc.sync.dma_start(out=t[:], in_=xt_d[kt * 128:(kt + 1) * 128,
                                                 0:chunk0])
            xts0.append(t)
        for kt in range(1, KT):
            nc.sync.dma_start(out=wt_sb[kt][:],
                              in_=wt_d[kt * 128:(kt + 1) * 128, :])
        eps_sb = const.tile([128, 1], f32, tag="eps")
        nc.vector.memset(eps_sb[:], EPS)
        if INJECT_MODE == "pe8":
            crow_sb = const.tile([1, 2 * N], f8, tag="crow8")
            nc.sync.dma_start(out=crow_sb[:], in_=crow_d[:])
            ones_sb = const.tile([1, 256], f8, tag="ones8")
            nc.sync.dma_start(out=ones_sb[:], in_=ones_d[:])
        else:
            crow_sb = const.tile([1, N], f16, tag="crow")
            nc.sync.dma_start(out=crow_sb[:], in_=crow_d[:])
            ones_sb = const.tile([1, 128], f16, tag="ones")
            nc.vector.memset(ones_sb[:], 1.0)
        if apply_affine:
            gam_sb = const.tile([128, N], f32, tag="gam")
            nc.sync.dma_start(out=gam_sb[:], in_=gam_d[:])
            bet_sb = const.tile([128, N], f32, tag="bet")
            nc.sync.dma_start(out=bet_sb[:], in_=bet_d[:])

        state = {}
        xts_cur = [None]

        def emit_main(mt):
            sc, loc = divmod(mt, tpc)
            if loc == 0:
                if mt == 0:
                    xts_cur[0] = xts0
                else:
                    xts = []
                    for kt in range(KT):
                        t = xpool.tile([128, chunk], f16, tag="xts")
                        nc.sync.dma_start(
                            out=t[:],
                            in_=xt_d[kt * 128:(kt + 1) * 128,
                                     sc * chunk:(sc + 1) * chunk])
                        xts.append(t)
                    xts_cur[0] = xts
            xts = xts_cur[0]
            ph = pp.tile([128, N], f32, tag="ph")
            for kt in range(KT):
                lhsT = xts[kt][:, loc * 128:(loc + 1) * 128]
                for h in range(2):
                    nc.tensor.matmul(
                        ph[:, 512 * h:512 * (h + 1)], lhsT,
                        wt_sb[kt][:, 512 * h:512 * (h + 1)],
                        start=(kt == 0), stop=False)
            # bias-minus-groupmean row injected last (K=1 rank-1 update)
            if INJECT_MODE == "pe8":
                for h in range(2):
                    nc.tensor.matmul(
                        ph[:, 512 * h:512 * (h + 1)],
                        ones_sb[:].rearrange("p (two f) -> p two f", two=2),
                        crow_sb[:, 1024 * h:1024 * (h + 1)].rearrange(
                            "p (two f) -> p two f", two=2),
                        start=False, stop=True,
                        perf_mode=mybir.MatmulPerfMode.DoubleRow)
            else:
                for h in range(2):
                    nc.tensor.matmul(ph[:, 512 * h:512 * (h + 1)],
                                     ones_sb[:],
                                     crow_sb[:, 512 * h:512 * (h + 1)],
                                     start=False, stop=True)
            state[mt] = ph

        def emit_epi(mt):
            ph = state.pop(mt)
            ysq = mid.tile([128, N], f16, tag="ysq")
            nc.scalar.square(ysq[:], ph[:])
            Q = small.tile([128, NG], f16, tag="Q")
            with nc.allow_low_precision("groupsum of 32 fp16 squares"):
                if GFOLD:
                    # Pool engine folds 32-wide groups to 8-wide first
                    y2 = mid.tile([128, N // 2], f16, tag="y2")
                    lo = ysq[:, 0:N]
                    hi = ysq[:, GS // 2:N]
                    in0 = bass.AP(tensor=lo.tensor, offset=lo.offset,
                                  ap=[lo.ap[0], [GS, NG], [1, GS // 2]])
                    in1 = bass.AP(tensor=hi.tensor, offset=hi.offset,
                                  ap=[hi.ap[0], [GS, NG], [1, GS // 2]])
                    nc.gpsimd.tensor_tensor(out=y2[:], in0=in0, in1=in1,
                                            op=Alu.add)
                    nc.vector.tensor_reduce(
                        out=Q[:],
                        in_=y2[:].rearrange("p (g e) -> p g e", e=GS // 2),
                        axis=mybir.AxisListType.X, op=Alu.add)
                else:
                    nc.vector.tensor_reduce(
                        out=Q[:],
                        in_=ysq[:].rearrange("p (g e) -> p g e", e=GS),
                        axis=mybir.AxisListType.X, op=Alu.add)
            # rstd = rsqrt(Q/32 + eps) in one ACT op. The wrapper refuses
            # Rsqrt (low-precision LUT) but tolerance here is 2e-2, so emit
            # the instruction directly; scale+bias fold in.
            r = small.tile([128, NG], f32, tag="r")
            if RSQRT:
                ins_ = [nc.scalar.lower_ap(Q[:]),
                        nc.scalar.lower_ap(eps_sb[:]),
                        mybir.ImmediateValue(dtype=f32, value=1.0 / GS),
                        mybir.ImmediateValue(dtype=f32, value=0.0)]
                nc.scalar.add_instruction(mybir.InstActivation(
                    name=nc.get_next_instruction_name(),
                    func=mybir.ActivationFunctionType.Rsqrt,
                    ins=ins_, outs=[nc.scalar.lower_ap(r[:])]))
            else:
                s = small.tile([128, NG], f32, tag="s")
                nc.scalar.activation(
                    out=s[:], in_=Q[:],
                    func=mybir.ActivationFunctionType.Sqrt,
                    bias=eps_sb[:], scale=1.0 / GS)
                nc.vector.reciprocal_approx_fast(r[:], s[:])
            o = outp.tile([128, N], f16, tag="o")
            lo, hi = (-1e30, 1e30) if apply_affine else (-HT, HT)
            for h in range(2):
                rh = r[:, 16 * h:16 * (h + 1)]
                rb = bass.AP(tensor=rh.tensor, offset=rh.offset,
                             ap=[rh.ap[0], rh.ap[1], [0, GS]])
                nc.vector._custom_dve(
                    ops["apply"],
                    out=o[:, 512 * h:512 * (h + 1)].rearrange(
                        "p (g e) -> p g e", e=GS),
                    in0=ph[:, 512 * h:512 * (h + 1)].rearrange(
                        "p (g e) -> p g e", e=GS),
                    in1=rb, s0=lo, s1=hi)
            if apply_affine:
                nc.vector.tensor_mul(o[:], o[:], gam_sb[:])
                nc.vector.tensor_add(o[:], o[:], bet_sb[:])
                nc.vector.tensor_scalar(
                    out=o[:], in0=o[:], scalar1=-HT, scalar2=HT,
                    op0=Alu.max, op1=Alu.min)
            nc.sync.dma_start(out=out_d[mt * 128:(mt + 1) * 128, :], in_=o[:])

        for mt in range(n_tiles):
            emit_main(mt)
            if mt >= 1:
                emit_epi(mt - 1)
        emit_epi(n_tiles - 1)

    nc.finalize()
    return nc


def _prep_host(weight, bias):
    f16 = np.float16
    w = weight.astype(np.float64).reshape(NG, GS, K)
    wp = (w - w.mean(axis=1, keepdims=True)).reshape(N, K)
    wt_h = np.ascontiguousarray(wp.T.astype(f16))  # [K, N]
    b = bias.astype(np.float64).reshape(NG, GS)
    c = (b - b.mean(axis=1, keepdims=True)).reshape(N)
    if INJECT_MODE == "pe8":
        import ml_dtypes
        f8 = ml_dtypes.float8_e4m3fn
        # DoubleRow layout [K=1, two, F]: sub-row 0 carries c, sub-row 1 is 0
        crow_h = np.zeros((1, 2 * N), dtype=f8)
        crow_h[0, 0:512] = c[0:512].astype(f8)
        crow_h[0, 1024:1536] = c[512:1024].astype(f8)
        ones_h = np.zeros((1, 256), dtype=f8)
        ones_h[0, 0:128] = f8(1.0)
        return wt_h, crow_h, ones_h
    crow_h = np.ascontiguousarray(c[None, :].astype(f16))
    return wt_h, crow_h, None


def run(x, weight, bias, gamma, beta, m_loc=None, trace=False):
    from concourse.bass_utils import run_bass_kernel_spmd

    f16 = np.float16
    x = np.asarray(x, dtype=np.float32)
    weight = np.asarray(weight, dtype=np.float32)
    bias = np.asarray(bias, dtype=np.float32)
    gamma = np.asarray(gamma, dtype=np.float32)
    beta = np.asarray(beta, dtype=np.float32)

    m_total = x.shape[0]
    if m_loc is None:
        m_loc = m_total // N_CORES
    assert m_total == m_loc * N_CORES

    apply_affine = not (np.all(gamma == 1.0) and np.all(beta == 0.0))
    nc = build(m_loc, apply_affine)
    wt_h, crow_h, ones_h = _prep_host(weight, bias)

    in_maps = []
    for c in range(N_CORES):
        m = {
            "xt": np.ascontiguousarray(
                x[c * m_loc:(c + 1) * m_loc, :].T.astype(f16)),
            "wt": wt_h,
        }
        if INJECT_MODE == "pe8":
            m["crow8"] = crow_h
            m["ones8"] = ones_h
        else:
            m["crow"] = crow_h
        if apply_affine:
            m["gam"] = np.ascontiguousarray(np.broadcast_to(gamma, (128, N)))
            m["bet"] = np.ascontiguousarray(np.broadcast_to(beta, (128, N)))
        in_maps.append(m)

    res = run_bass_kernel_spmd(nc, in_maps, list(range(N_CORES)), trace=trace)
    out = np.concatenate([res.results[c]["out"] for c in range(N_CORES)],
                         axis=0).astype(np.float32)
    return out, res


def kernel(x, weight, bias, gamma, beta):
    out, _ = run(x, weight, bias, gamma, beta)
    return out
